# revision 3
# baseline (speedup 1.0000x reference)
"""CTRNN forward kernel v3 for Trainium2 (8 NeuronCores).

Model (per step t):
    pre = x_t @ w_in^T + b_in + h @ w_hh^T + b_hh + sigma * n_t
    h'  = (1-a)*h + a*relu(pre)

For w_hh = d*I the recurrence is elementwise:
    h' = max(coef_a*h + v, coef_c*h)   with v = a*(x w^T + b + sigma n)

v3: 2D sharding = 4 time-shards x 2 batch-shards. The step map is a
0.9-contraction, so each time shard starts 48 steps early from h=0 and
the initial-state error decays below 1e-3 by its owned range (warmup).
This cuts the serial DVE chain from 1024 to 304 steps/core while
quadrupling the per-step op width (256), amortizing the fixed ~60ns
SBUF-access charge per DVE op: 304 * (60.4 + 256*1.042) ~= 99us.

PE uses fp8e4m3 DoubleRow matmuls (2 contraction rows/pass, 0.5
cycles/row). Pair-split quantization keeps bf16-level accuracy with 3
fp8 matmuls (vs 4x cost of 1 bf16 matmul per 128-contraction):
    u ~= x_hi@V0 + x_lo16@(V0/16) + x_hi@(W - V0)
Weights are pre-scaled by 256 (power of 2) to avoid fp8 denormals; the
PSUM evacuation applies 32/256 so v_sb holds 32*v in bf16, the noise
rides as fp8 at 32x scale (SWDGE accumulate-DMA), and the DVE step op
applies the final 1/32 via its imm2 constant.

Rings: SP = x_hi+x_lo loads, Pool/SWDGE = noise accum + out store
(bf16 cast), ACT = weights/h0 + PSUM evacuation. All three transfer
streams overlap in the DMA fabric.

Per 8-step block: PE 24 DoubleRow matmuls (2.56us), ACT evac (1.89us),
Pool 2 SWDGE (2.27us), DVE 8 fused step ops (2.62us, critical).
"""

import os
import sys

import numpy as np

for _p in ("/opt/trn_rl_repo", os.path.expanduser("~/.axon_site/_ro/trn_rl_repo")):
    if os.path.isdir(_p) and _p not in sys.path:
        sys.path.insert(0, _p)

S, B, I, H = 1024, 128, 512, 512
TAU, DT = 100.0, 20.0
ALPHA = DT / TAU  # 0.2
SIGMA_REC = 0.05
SIGMA = float(np.sqrt(2.0 / ALPHA) * SIGMA_REC)

NCORES = 8
TSH = 4  # time shards
BSH = 2  # batch shards
BL = B // BSH  # 64 batch rows per core
OWN = S // TSH  # 256 owned steps per time shard
WARM = 48  # warmup steps (0.9^48 ~ 6e-3 contraction of h-init error)
STEPS = OWN + WARM  # 304 computed steps per core
SPB = 8  # steps per block
NB = STEPS // SPB  # 38 blocks
KC = I // 128  # 4 contraction chunks
HC = H // 128  # 4 hidden chunks
FB = SPB * BL  # 512 matmul free elems per block (sl, b)
VW = HC * BL  # 256: step op width (c, b)
NMM = 3  # fp8 matmul variants (hi@V0, lo@V1, hi@V2)
SW = 256.0  # weight prescale (power of 2)
NS = 32.0  # noise / v_sb prescale (power of 2)
EVAC_SCALE = NS / SW  # applied at PSUM evacuation
IMM2 = 1.0 / NS  # applied to v inside the DVE step op

_PROGRAM_CACHE: dict = {}
_CTRNN_OP = None


def _get_ctrnn_dve_op():
    """Fused DVE op: out = max(in0*s0 + in1*imm2, in0*s1)."""
    global _CTRNN_OP
    if _CTRNN_OP is not None:
        return _CTRNN_OP
    import concourse.dve_ops as dve_ops
    from concourse.dve_spec import Spec, Src0, Src1, _has_src1, lower, maxx
    from concourse.dve_spec import C0, C1, C2
    from concourse.dve_uop import DveOpSpec

    name = "CTRNN_STEP3_ANT"
    spec = Spec(
        body=maxx(Src0 * C0 + Src1 * C2, Src0 * C1),
        reference=lambda in0, in1, s0, s1, imm2: np.maximum(
            in0.astype(np.float32) * s0 + in1.astype(np.float32) * imm2,
            in0.astype(np.float32) * s1,
        ).astype(np.float32),
    )
    row = max(dve_ops._SUB_OPCODE_FOR_NAME.values()) + 1
    assert row < 0x20
    dve_ops._SUB_OPCODE_FOR_NAME[name] = row
    shas = {}
    for ver in ("v3", "v4"):
        try:
            shas[ver] = DveOpSpec(
                name=name, opcode=row, uops=lower(spec, ver=ver),
                rd1_en=_has_src1(spec),
            ).sha(ver)
        except Exception:
            pass
    op = dve_ops.DveOp(name, spec, subdim=False, uops_sha=shas)
    dve_ops.OPS.append(op)
    dve_ops.CUSTOM_DVE_SPECS[name] = spec
    _CTRNN_OP = op
    return op


def _build_program(n_blocks: int, coef_a: float, coef_c: float):
    import concourse.bacc as bacc
    import concourse.mybir as mybir
    from concourse import tile

    f32 = mybir.dt.float32
    bf16 = mybir.dt.bfloat16
    fp8 = mybir.dt.float8e4
    copy_fn = mybir.ActivationFunctionType.Copy
    add = mybir.AluOpType.add
    double_row = mybir.MatmulPerfMode.DoubleRow

    nc = bacc.Bacc(
        "TRN2",
        target_bir_lowering=False,
        debug=False,
        num_devices=NCORES,
    )

    xh_d = nc.dram_tensor("xh_t", [KC, 128, n_blocks, FB], fp8, kind="ExternalInput")
    xl_d = nc.dram_tensor("xl_t", [KC, 128, n_blocks, FB], fp8, kind="ExternalInput")
    # w_t[p, m, kcp, hc, kt, col] = Vm[hc*128+col, kcp*256 + kt*128 + p]
    w_d = nc.dram_tensor("w_t", [128, NMM, 2, HC, 2, 128], fp8, kind="ExternalInput")
    n_d = nc.dram_tensor("n_t", [n_blocks, 128, SPB * VW], fp8, kind="ExternalInput")
    h0_d = nc.dram_tensor("h0l", [128, VW], f32, kind="ExternalInput")
    o_d = nc.dram_tensor("out_l", [n_blocks, 128, SPB * VW], bf16, kind="ExternalOutput")

    WFREE = NMM * 2 * HC * 2 * 128  # 6144

    HSL = SPB // 2  # 4 steps per half
    HV = HSL * VW  # 1024 v elems per half

    with tile.TileContext(nc) as tc:
        with (
            tc.tile_pool(name="const", bufs=1) as cpool,
            tc.tile_pool(name="xp", bufs=3) as xpool,
            tc.tile_pool(name="pp", bufs=2, space="PSUM") as ppool,
            tc.tile_pool(name="vp", bufs=4) as vpool,
            tc.tile_pool(name="op", bufs=3) as opool,
        ):
            # weights + h0 on the ACT ring, overlapping the first x load (SP)
            w_sb = cpool.tile([128, WFREE], fp8)
            wh = WFREE // 2
            for hf in range(2):
                nc.scalar.dma_start(
                    out=w_sb[:, hf * wh : (hf + 1) * wh],
                    in_=w_d.ap()
                    .rearrange("p m kcp hc kt c -> p (m kcp hc kt c)")[
                        :, hf * wh : (hf + 1) * wh
                    ],
                )
            h0_sb = cpool.tile([128, VW], f32)
            nc.scalar.dma_start(out=h0_sb[:], in_=h0_d.ap())

            w_view = w_sb[:].rearrange(
                "p (m kcp hc kt c) -> p m kcp hc kt c", m=NMM, kcp=2, hc=HC, kt=2
            )

            prev = h0_sb[:]
            prev_o = None
            for blk in range(n_blocks):
                # ---- x block loads (SP ring)
                xh_sb = xpool.tile([128, KC * FB], fp8)
                nc.sync.dma_start(
                    out=xh_sb[:].rearrange("p (kc f) -> p kc f", kc=KC),
                    in_=xh_d.ap()[:, :, blk, :].rearrange("kc p f -> p kc f"),
                )
                xl_sb = xpool.tile([128, KC * FB], fp8)
                nc.sync.dma_start(
                    out=xl_sb[:].rearrange("p (kc f) -> p kc f", kc=KC),
                    in_=xl_d.ap()[:, :, blk, :].rearrange("kc p f -> p kc f"),
                )
                xh_view = xh_sb[:].rearrange("p (kc f) -> p kc f", kc=KC)
                xl_view = xl_sb[:].rearrange("p (kc f) -> p kc f", kc=KC)

                # ---- store the previous block (ACT ring) before issuing the
                # next evacs: a DMA holds its sequencer through its waits, so
                # the store (gated on the DVE chain) must sit behind nothing
                if prev_o is not None:
                    nc.scalar.dma_start(out=o_d.ap()[blk - 1], in_=prev_o[:])

                # ---- per step-half: 24 DoubleRow matmuls -> evac -> noise
                # accum. Half granularity cuts the producer->DVE latency so
                # the accum-DMA (dge 650 + xfer 730 + sem 900) lands in time.
                ps = ppool.tile([128, HC * FB], f32)
                ps_view = ps[:].rearrange("p (c sl b) -> p c sl b", c=HC, b=BL)
                v_sb = vpool.tile([128, SPB * VW], bf16)
                for h in range(2):
                    fsl = slice(h * FB // 2, (h + 1) * FB // 2)
                    for hc in range(HC):
                        idx = 0
                        for m, xv in ((0, xh_view), (1, xl_view), (2, xh_view)):
                            for kcp in range(2):
                                nc.tensor.matmul(
                                    out=ps[
                                        :,
                                        hc * FB + h * FB // 2 : hc * FB
                                        + (h + 1) * FB // 2,
                                    ],
                                    lhsT=w_view[:, m, kcp, hc],
                                    rhs=xv[:, 2 * kcp : 2 * kcp + 2, fsl],
                                    start=(idx == 0),
                                    stop=(idx == 5),
                                    perf_mode=double_row,
                                )
                                idx += 1
                    # evacuation (ACT): (c, sl, b) -> (sl, c, b), scale 32/256
                    nc.scalar.activation(
                        out=v_sb[:, h * HV : (h + 1) * HV].rearrange(
                            "p (sl c b) -> p c sl b", c=HC, b=BL
                        ),
                        in_=ps_view[:, :, h * HSL : (h + 1) * HSL, :],
                        func=copy_fn,
                        scale=EVAC_SCALE,
                    )
                    # noise+bias accumulate (SWDGE fp8 add, 32x prescaled)
                    nc.gpsimd.dma_start(
                        out=v_sb[:, h * HV : (h + 1) * HV],
                        in_=n_d.ap()[blk, :, h * HV : (h + 1) * HV],
                        accum_op=add,
                    )

                # ---- recurrence: one fused DVE op per step (bf16 state)
                o_sb = opool.tile([128, SPB * VW], bf16)
                for st in range(SPB):
                    osl = o_sb[:, st * VW : (st + 1) * VW]
                    nc.vector._custom_dve(
                        _get_ctrnn_dve_op(), out=osl, in0=prev,
                        in1=v_sb[:, st * VW : (st + 1) * VW],
                        s0=coef_a, s1=coef_c, imm2=IMM2,
                    )
                    prev = osl
                prev_o = o_sb

            nc.scalar.dma_start(out=o_d.ap()[n_blocks - 1], in_=prev_o[:])

    nc.finalize()
    return nc


def _get_program(n_blocks, coef_a, coef_c):
    key = (n_blocks, coef_a, coef_c)
    if key not in _PROGRAM_CACHE:
        _PROGRAM_CACHE[key] = _build_program(n_blocks, coef_a, coef_c)
    return _PROGRAM_CACHE[key]


def _f8():
    import ml_dtypes

    return np.dtype(ml_dtypes.float8_e4m3)


def _pack_weights(w_in):
    """Three fp8 matrices for the pair-split matmul, packed for DoubleRow.

    Returns w_t[p, m, kcp, hc, kt, col] = Vm[hc*128+col, kcp*256+kt*128+p].
    """
    f8 = _f8()
    w256 = (SW * ALPHA) * w_in.astype(np.float32)  # [H, I]
    v0 = w256.astype(f8)
    v0f = v0.astype(np.float32)
    v1 = (v0f / 16.0).astype(f8)
    v2 = (w256 - v0f).astype(f8)
    pack = np.stack([v0, v1, v2])  # [3, H, I]
    # [3, HC, col(128), kcp(2), kt(2), p(128)] -> [p, m, kcp, hc, kt, col]
    w_t = pack.reshape(NMM, HC, 128, 2, 2, 128).transpose(5, 0, 3, 1, 4, 2)
    return np.ascontiguousarray(w_t)


def _core_shards():
    return [(c // BSH, c % BSH) for c in range(NCORES)]  # (tau, beta)


def _host_inputs(x, noise, w_in, b_in, b_hh, h0):
    """Per-core input dicts (all layout + quantization work on the host)."""
    f8 = _f8()
    w_t = _pack_weights(w_in)

    xh_full = x.astype(f8)
    xl_full = ((x - xh_full.astype(np.float32)) * 16.0).astype(f8)
    bias = (NS * ALPHA) * (b_in + b_hh).astype(np.float32)
    nh_full = ((NS * ALPHA * SIGMA) * noise.astype(np.float32) + bias).astype(f8)

    in_maps = []
    for tau, beta in _core_shards():
        s0 = 0 if tau == 0 else tau * OWN - WARM
        ss = slice(s0, s0 + STEPS)
        bs = slice(beta * BL, (beta + 1) * BL)

        def pack_x(xf):
            # x_c[kc, p, blk, (sl, b)] = xf[s0+blk*SPB+sl, b, kc*128+p]
            xc = (
                xf[ss, bs, :]
                .reshape(NB, SPB, BL, I)
                .transpose(3, 0, 1, 2)
                .reshape(KC, 128, NB, FB)
            )
            return np.ascontiguousarray(xc)

        # n_c[blk, p, (sl, c, b)] = nh[s0+blk*SPB+sl, b, c*128+p]
        n_c = (
            nh_full[ss, bs, :]
            .reshape(NB, SPB, BL, HC, 128)
            .transpose(0, 4, 1, 3, 2)
            .reshape(NB, 128, SPB * VW)
        )
        if tau == 0:
            h0_l = (
                h0[bs].astype(np.float32).reshape(BL, HC, 128).transpose(2, 1, 0)
            )
            h0_l = np.ascontiguousarray(h0_l.reshape(128, VW))
        else:
            h0_l = np.zeros((128, VW), dtype=np.float32)
        in_maps.append(
            {
                "xh_t": pack_x(xh_full),
                "xl_t": pack_x(xl_full),
                "w_t": w_t,
                "n_t": np.ascontiguousarray(n_c),
                "h0l": h0_l,
            }
        )
    return in_maps


def _gather_output(results):
    out = np.empty((S, B, H), dtype=np.float32)
    for c, (tau, beta) in enumerate(_core_shards()):
        o = np.asarray(results[c]["out_l"], dtype=np.float32)
        # o[blk, p, (sl, c, b)] -> steps[s, b, h=(hc, p)]
        o = (
            o.reshape(NB, 128, SPB, HC, BL)
            .transpose(0, 2, 4, 3, 1)
            .reshape(STEPS, BL, H)
        )
        off = 0 if tau == 0 else WARM
        out[tau * OWN : (tau + 1) * OWN, beta * BL : (beta + 1) * BL, :] = o[
            off : off + OWN
        ]
    return out


def _numpy_fallback(x, noise, w_in, b_in, w_hh, b_hh, h0):
    h = h0.astype(np.float32).copy()
    out = np.empty((S, B, H), dtype=np.float32)
    one_minus_a = np.float32(1.0 - ALPHA)
    a = np.float32(ALPHA)
    sg = np.float32(SIGMA)
    for t in range(S):
        pre = x[t] @ w_in.T + b_in + h @ w_hh.T + b_hh + sg * noise[t]
        h = h * one_minus_a + np.maximum(pre, 0) * a
        out[t] = h
    return out


def kernel(x, noise, w_in, b_in, w_hh, b_hh, h0):
    x = np.asarray(x, dtype=np.float32)
    noise = np.asarray(noise, dtype=np.float32)
    w_in = np.asarray(w_in, dtype=np.float32)
    b_in = np.asarray(b_in, dtype=np.float32)
    w_hh = np.asarray(w_hh, dtype=np.float32)
    b_hh = np.asarray(b_hh, dtype=np.float32)
    h0 = np.asarray(h0, dtype=np.float32)

    d = np.diagonal(w_hh)
    uniform_diag = np.all(w_hh == np.diag(d)) and np.all(d == d[0])
    if not uniform_diag:
        return _numpy_fallback(x, noise, w_in, b_in, w_hh, b_hh, h0)

    dval = float(d[0])
    coef_a = (1.0 - ALPHA) + ALPHA * dval  # 0.9 for d=0.5
    coef_c = 1.0 - ALPHA  # 0.8

    from concourse.bass_utils import run_bass_kernel_spmd

    nc = _get_program(NB, coef_a, coef_c)
    in_maps = _host_inputs(x, noise, w_in, b_in, b_hh, h0)
    res = run_bass_kernel_spmd(nc, in_maps, list(range(NCORES)))
    return _gather_output(res.results)
